# revision 41
# baseline (speedup 1.0000x reference)
"""Single-head attention on 8 TRN2 NeuronCores, data-parallel over batch.

Per core (one batch element b):
  x_b [2048, 768] -> Q = x Wq, K = x Wk, V = x Wv (head 64)
  scores^T[k, q] per (q-half, k-tile); E = exp(scores/8); U^T = [V|1]^T E
  (ones row accumulates the softmax denominator); transpose + normalize.

Structure (v1 98 us -> v5 ~66 -> v6 ~65 at nominal clock; chip clock and
x-DMA drain vary run to run: exec spans ~65-70 normal-clock, +20% when the
chip throttles; PV-matmul slice dur 216 vs 259 ns tells which you got):
  - x is transposed AND cast to bf16 on the HOST; device receives x^T
    strip-major so each 512-seq strip is one contiguous 6 KB/partition DMA.
  - x strips trigger from GpSimd/SWDGE FIRST (order 0..3): SDMA queues are
    FIFO per queue, so strip 0 drains ahead of 1-3. Weights ride the Sync
    queue set in parallel. The ident builds AFTER the triggers; warmup uses
    the zeroed warm_src as lhsT so it starts ~7.7 us without waiting.
  - PE HAM warmup: the clock gate opens after a ~fully-busy 4096-cycle
    window and a >~1 us gap restarts it (short ~0.6 us gaps are survivable),
    so a fine-grained N=128 tail butts against strip 0's ~13.5-15 us
    arrival (tail self-compensates: hot chip -> shorter tail, but hot chip
    also drains DMA faster).
  - projections: even strips use wqk=[Wq|Wk], odd strips wkq=[Wk|Wq], so Q
    always lands in its NATIVE partition half (q0 lo / q1 hi) and only K
    needs replicating to the other half. That replication is an
    identity-weighted PE matmul through psum + engine copy (emit_move), NOT
    an SBUF->SBUF DMA: swap DMAs queue behind the x-stream descriptors on
    the shared DMA engines and put 1-4 us of run-to-run variance on the
    first score matmul (v5's worst head stall).
  - head order: qk[0], qk[1] (v[0] between them fills the strip-1 DMA
    window), then both K-replications into ps_warm's idle second bank; all
    head psum->SBUF copies split K->scalar / Q->vector to run in parallel.
    First scores ~18.5 us (v5: 20.7). Remaining v/qk projections + V
    transposes are PE fillers keyed through the loop's first ~10 gi.
  - phase 3 runs in BLOCKS of 2 kt: [scores(k) scores(k+1)] then
    [PV(k-2) PV(k-1)] — the PE array-mode reconfig between the split-tile
    score pairs (64,128) and full-height PV (128,128) happens twice per
    block instead of four times, and the second score pair's LDWEIGHTS
    hides under the first's stream (measured 213 ns vs 318 for pair 1).
    Steady block = 1514 ns = 757/kt (v5: 872/kt). An alternating
    PV-first/scores-first order with lookahead 4 halves reconfigs again
    but turns the loop exp-bound — measured slower.
  - scalar/DVE exps alternate by kt parity (whole [128,2,512] tiles,
    1.08/1.22 us) — per-block exp floor 1.22 < PE 1.51, so the loop stays
    PE-bound. A j-split (both engines every kt) halves et latency but the
    extra instruction startups push exp throughput into the red.
  - exp on DVE is a custom one-instruction op: exp(s/8) = q^4 with
    q = c0+s(c1+s(c2+s c3)) fit for exp(s/32) (~2.5e-3 rel err).
  - PV accumulators: ONE [65,2,512] 2-bank psum tile per qh (pool bufs=1);
    at the qh boundary both j-copies fire immediately on opposite engines
    so qh1's first PV waits only ~one copy. N=1024 single-matmul PV fails
    the ISA check s3d3_mm_num_elements (moving free capped at 512).
  - finalize per (qh, j): 4 PE transposes into ONE psum tile [128,4,65],
    one batched reciprocal [128,4], normalize muls on scalar (qh0) /
    split (qh1); qh0's emission spread through the next qh's loop.
  - out stays [p, a, h] (s = a*128 + p) on the device so each finalize DMA
    is 128 x 1 KB contiguous descriptors (v5 wrote 512 x 256 B scattered
    rows; the teardown's queue drain shrank ~1.2 us); kernel() untangles.
  - gpsimd CANNOT read psum (BIR verifier) — all psum->SBUF drains must go
    through scalar/DVE.
"""

import numpy as np
import ml_dtypes

import concourse.tile as tile
from concourse import bacc, mybir
from concourse import dve_ops as _dve_ops
from concourse.bass_utils import run_bass_kernel_spmd
from concourse.dve_spec import (
    C0, C1, C2, C3, Spec, Src0, _spill_c3_to_src1, lower, sq,
)
from concourse.dve_table_gen import dve_ver_for
from concourse.dve_uop import DveOpSpec
from concourse.masks import make_identity

B, S, D, H = 8, 2048, 768, 64
P = 128
NT = S // P      # 16 k-tiles
NCH = D // P     # 6 emb chunks
QC = 512         # q-chunk width (one psum bank of f32)
NSTRIP = S // QC  # 4 strips
N_CORES = 8
LOOKAHEAD = 2    # scores run this many kt ahead of PV
F32 = mybir.dt.float32
BF16 = mybir.dt.bfloat16
EXP = mybir.ActivationFunctionType.Exp
SCALE = float(1.0 / np.sqrt(H))

# cubic fit of exp(v) on v = s/32 in [-0.6, 0.6]; exp(s/8) = q(s)^4.
_EXPQ_C = (0.99941146373748779, 0.031295426189899445,
           5.0254801753908396e-04, 4.9950904040038586e-06)
_EXPQ_NAME = "EXP_QUARTIC_ANT"


def _register_exp_op():
    for op in _dve_ops.OPS:
        if op.name == _EXPQ_NAME:
            return op
    body = _spill_c3_to_src1(
        sq(sq(((Src0 * C2 + C1) * Src0 + C0) * Src0 + C3)))

    def _ref(in0, in1, s0, s1, imm2):
        x = in0.astype(np.float32)
        c0 = np.asarray(in1, np.float32).reshape(x.shape[0], *([1] * (x.ndim - 1)))
        q = ((x * imm2 + s1) * x + s0) * x + c0
        return (q * q) * (q * q)

    spec = Spec(body=body, reference=_ref)
    row = max(_dve_ops._SUB_OPCODE_FOR_NAME.values()) + 1
    assert row < 0x20
    _dve_ops._SUB_OPCODE_FOR_NAME[_EXPQ_NAME] = row
    ver = dve_ver_for("TRN2")
    uops = lower(spec, ver=ver)
    sha = DveOpSpec(name=_EXPQ_NAME, opcode=row, uops=uops, rd1_en=True).sha(ver)
    op = _dve_ops.DveOp(_EXPQ_NAME, spec, subdim=False, uops_sha={ver: sha})
    _dve_ops.OPS.append(op)
    _dve_ops.CUSTOM_DVE_SPECS[_EXPQ_NAME] = spec
    return op


_EXPQ_OP = _register_exp_op()


def build_kernel():
    nc = bacc.Bacc("TRN2", num_devices=N_CORES)
    x_ext = nc.declare_dram_parameter("x", [P, NSTRIP * NCH * QC], BF16,
                                      isOutput=False)
    wqk_ext = nc.declare_dram_parameter("wqk", [P, NCH * P], BF16,
                                        isOutput=False)
    wkq_ext = nc.declare_dram_parameter("wkq", [P, NCH * P], BF16,
                                        isOutput=False)
    wv_ext = nc.declare_dram_parameter("wv", [P, NCH * H], BF16,
                                       isOutput=False)
    # out stays partition-major on the device ([p, a, h] with s = a*128 + p):
    # each finalize DMA writes 1 KB/partition contiguous (128 descriptors)
    # instead of 512 x 256 B scattered rows; the host untangles.
    # bf16 out: halves the final DMA + teardown queue drain; adds ~0.1%
    # rounding against a 2e-2 gate (kernel() casts back to f32)
    out_ext = nc.declare_dram_parameter("out", [P, (S // P) * H], BF16,
                                        isOutput=True)

    with tile.TileContext(nc) as tc:
        _body(nc, tc, x_ext, wqk_ext, wkq_ext, wv_ext, out_ext)
    nc.compile()
    return nc


def _body(nc, tc, x_ext, wqk_ext, wkq_ext, wv_ext, out_ext):
    with (
        tc.tile_pool(name="singles", bufs=1) as singles,
        tc.tile_pool(name="et", bufs=4) as et_pool,
        tc.tile_pool(name="fin", bufs=4) as fin_pool,
        tc.tile_pool(name="ps_s", bufs=3, space="PSUM") as ps_s,
        tc.tile_pool(name="ps_u", bufs=1, space="PSUM") as ps_u_pool,
    ):
        # --- trigger ALL x-strip DMAs from gpsimd FIRST (order 0..3): the
        # SDMA queues service descriptors FIFO per queue, so strip 0 drains
        # at full bandwidth while 1-3 follow behind it; triggering before
        # the ident build shaves ~0.7 us off strip 0's arrival. (v4 had
        # strips 2-3 + weights triggered first, which starved strip 0
        # until 9+ us and stalled/re-throttled the PE.)
        x_r = x_ext.rearrange("p (t c s) -> p t c s", c=NCH, s=QC)
        xt_sb = singles.tile([P, NSTRIP, NCH, QC], BF16, tag="xt_sb")
        # weights on the sync queue: they land ~3 us (before strip 0 is
        # drained) WITHOUT their descriptors cutting ahead of strip 0's.
        wqk_sb = singles.tile([P, NCH, P], BF16, tag="wqk_sb")
        nc.sync.dma_start(out=wqk_sb, in_=wqk_ext.rearrange(
            "p (c m) -> p c m", m=P))
        # wkq = [Wk|Wq]: odd strips (the q1 halves) project with swapped
        # packing so their Q lands in partitions 64:128 natively — only
        # the K half needs replicating to the other half (emit_move).
        wkq_sb = singles.tile([P, NCH, P], BF16, tag="wkq_sb")
        nc.sync.dma_start(out=wkq_sb, in_=wkq_ext.rearrange(
            "p (c m) -> p c m", m=P))
        wv_sb = singles.tile([P, NCH, H], BF16, tag="wv_sb")
        nc.sync.dma_start(out=wv_sb, in_=wv_ext.rearrange(
            "p (c h) -> p c h", h=H))
        for sc in range(NSTRIP):
            nc.gpsimd.dma_start(out=xt_sb[:, sc], in_=x_r[:, sc])

        # warm_src zeroed on VECTOR and used as the warmup lhsT too, so the
        # PE warmup starts ~7.7 us — it doesn't wait for the gpsimd ident
        # build, which now runs after the triggers (ident is first needed
        # by emit_vtrans, well past strip 0's arrival).
        warm_src = singles.tile([P, QC], BF16, tag="warm_src")
        nc.vector.memset(warm_src, 0.0)
        ident_bf = singles.tile([P, P], BF16, tag="ident_bf")
        make_identity(nc, ident_bf)

        # HAM warmup across the DMA window. Any PE gap before the clock
        # gate opens restarts its fully-busy-window requirement, so the
        # tail is fine-grained (N=128) to butt up against strip 0's
        # arrival with minimal over- or under-shoot. Warmup starts ~7.7 us
        # (no ident wait); strip 0 is projection-ready ~13.5-15.3 us, so
        # the tail is sized to end ~14 (it runs ~71 ns/mm once the clock
        # gate opens mid-warmup; on a throttled chip it stretches, but so
        # does the DMA, which keeps the butt joint tight).
        ps_warm = ps_s.tile([P, 2, QC], F32, tag="ss", name="ps_warm")
        NWARM, NWARM_TAIL = 6, 46
        for i in range(NWARM):
            nc.tensor.matmul(ps_warm[:, 0, :], warm_src[:, 0:P], warm_src,
                             start=(i == 0), stop=False)
        for i in range(NWARM_TAIL):
            nc.tensor.matmul(ps_warm[:, 0, 0:P], warm_src[:, 0:P],
                             warm_src[:, 0:P],
                             start=False, stop=(i == NWARM_TAIL - 1))

        ident = singles.tile([P, P], F32)
        make_identity(nc, ident)
        c0_sb = singles.tile([P, 1], F32, tag="c0_sb")
        nc.vector.memset(c0_sb, _EXPQ_C[0])

        qk_sb = singles.tile([P, S], BF16, tag="qk_sb")
        qk_sw = singles.tile([P, S], BF16, tag="qk_sw")
        vt_sb = singles.tile([H, S], BF16, tag="vt_sb")   # V^T
        vp = singles.tile([P, NT, H + 1], BF16, tag="vp")  # V' = [V, 1]
        nc.vector.memset(vp[:, :, H:H + 1], 1.0)

        def emit_move(sc, out_ps, on_scalar):
            # Replicate strip sc's K^T into the opposite partition half of
            # qk_sw WITHOUT a DMA: an identity-weighted matmul shifts the
            # 64 partitions through the PE (~180 ns), then an engine copy
            # drains psum. SBUF->SBUF swap DMAs queue behind the x-stream
            # descriptors on the shared DMA engines, which put 1-4 us of
            # run-to-run variance on the first score matmul's critical path.
            sl = slice(sc * QC, (sc + 1) * QC)
            if sc % 2 == 0:  # K native in 64:128 -> replicate into 0:64
                nc.tensor.matmul(out_ps, ident_bf[H:P, H:P], qk_sb[H:P, sl],
                                 start=True, stop=True)
                dst = qk_sw[0:H, sl]
            else:            # K native in 0:64 -> replicate into 64:128
                nc.tensor.matmul(out_ps, ident_bf[0:H, 0:H], qk_sb[0:H, sl],
                                 start=True, stop=True)
                dst = qk_sw[H:P, sl]
            if on_scalar:
                nc.scalar.copy(out=dst, in_=out_ps)
            else:
                nc.vector.tensor_copy(dst, out_ps)

        def emit_qk(strips, on_scalar, split_copy=False, move=True):
            # chunk-outer over the given strips: each weight chunk is loaded
            # into the PE once and streams all of them (amortizes the
            # LDWEIGHTS serialization). Even strips use [Wq|Wk] (Q lands in
            # partitions 0:64), odd strips [Wk|Wq] (Q lands in 64:128) —
            # only the K half ever needs replication to the other half.
            sl = slice(strips[0] * QC, (strips[-1] + 1) * QC)
            swapped = strips[0] % 2 == 1
            w_sb = wkq_sb if swapped else wqk_sb
            psqk = ps_s.tile([P, 2, QC], F32, tag="ss", name="psqk")
            for c in range(NCH):
                for i, sc in enumerate(strips):
                    nc.tensor.matmul(psqk[:, i, :], w_sb[:, c],
                                     xt_sb[:, sc, c],
                                     start=(c == 0), stop=(c == NCH - 1))
            psl = psqk[:, 0:len(strips), :]
            k_half = slice(0, H) if swapped else slice(H, P)
            q_half = slice(H, P) if swapped else slice(0, H)
            if split_copy:
                # parallel engines: K half on scalar, Q half on vector
                nc.scalar.copy(out=qk_sb[k_half, sl], in_=psl[k_half])
                nc.vector.tensor_copy(qk_sb[q_half, sl], psl[q_half])
            elif on_scalar:
                nc.scalar.copy(out=qk_sb[:, sl], in_=psl)
            else:
                nc.vector.tensor_copy(qk_sb[:, sl], psl)
            if move:
                for sc in strips:
                    half = slice(0, H) if sc % 2 == 0 else slice(H, P)
                    emit_move(sc, psqk[half, 1, :], not on_scalar)

        def emit_v(strips, on_scalar):
            sl = slice(strips[0] * QC, (strips[-1] + 1) * QC)
            psvt = ps_s.tile([H, 2, QC], F32, tag="ss", name="psvt")
            for c in range(NCH):
                for i, sc in enumerate(strips):
                    nc.tensor.matmul(psvt[:, i, :], wv_sb[:, c],
                                     xt_sb[:, sc, c],
                                     start=(c == 0), stop=(c == NCH - 1))
            psl = psvt[:, 0:len(strips), :]
            if on_scalar:
                nc.scalar.copy(out=vt_sb[:, sl], in_=psl)
            else:
                nc.vector.tensor_copy(vt_sb[:, sl], psl)

        def emit_vtrans(sc):
            psvn = ps_s.tile([P, 4, H], BF16, tag="ss", name="psvn")
            for t in range(4):
                nc.tensor.transpose(
                    psvn[:, t, :],
                    vt_sb[:, sc * QC + t * P:sc * QC + (t + 1) * P],
                    ident_bf[:H, :H])
            nc.vector.tensor_copy(vp[:, sc * 4:(sc + 1) * 4, 0:H], psvn)

        # head: qk projections for strips 0 AND 1 first — the first score
        # matmul needs Q columns from both (the q-half pair spans q 0:1024).
        # The K-replication move-matmuls run after BOTH projections (their
        # source copies land while the other strip projects); their psum
        # lands in ps_warm's idle second bank. Copies split scalar/vector.
        # v[0] fills the PE window between qk[0]'s last matmul and strip 1's
        # arrival; all head psum->SBUF copies run split across both engines.
        emit_qk([0], True, split_copy=True, move=False)
        emit_v([0], True)
        emit_qk([1], False, split_copy=True, move=False)
        emit_move(0, ps_warm[0:H, 1, :], False)
        emit_move(1, ps_warm[H:P, 1, :], True)

        # remaining phase-2 work, spread inside the qh=0 loop (keyed so each
        # piece executes after its DMA lands and ~2 iterations before its
        # first consumer; all 4 strips are DMA-resident by ~2 iterations in)
        filler = {1: lambda: emit_vtrans(0), 2: lambda: emit_v([1], False),
                  3: lambda: emit_vtrans(1),
                  5: lambda: emit_qk([2], True), 6: lambda: emit_qk([3], False),
                  7: lambda: emit_v([2], True), 8: lambda: emit_v([3], False),
                  9: lambda: emit_vtrans(2), 10: lambda: emit_vtrans(3)}

        # --- phase 3: software-pipelined scores/exp -> PV
        def emit_scores_exp(qh, kt):
            ksl = slice(kt * P, (kt + 1) * P)
            et = et_pool.tile([P, 2, QC], BF16, name="et")
            ps = ps_s.tile([P, 2, QC], F32, tag="ss", name="ps")
            q0 = (qh * 2) * QC
            q1 = (qh * 2 + 1) * QC
            # K^T for this kt is native in one partition half and swapped
            # into the other; Q^T is always native (q0 lo / q1 hi).
            k_swapped = (kt // 4) % 2 == 1
            k_lo = (qk_sb if k_swapped else qk_sw)[0:H, ksl]
            k_hi = (qk_sw if k_swapped else qk_sb)[H:P, ksl]
            nc.tensor.matmul(ps[:, 0, :], k_lo, qk_sb[0:H, q0:q0 + QC],
                             start=True, stop=True)
            nc.tensor.matmul(ps[:, 1, :], k_hi, qk_sb[H:P, q1:q1 + QC],
                             start=True, stop=True)
            if kt % 2 == 0:
                nc.scalar.activation(et, ps, EXP, scale=SCALE)
            else:
                nc.vector._custom_dve(
                    _EXPQ_OP, out=et, in0=ps, in1=c0_sb,
                    s0=_EXPQ_C[1], s1=_EXPQ_C[2], imm2=_EXPQ_C[3])
            return et

        def emit_pv(psum_u, kt, et):
            # (a single N=1024 matmul fails the ISA check s3d3_mm_num_elements
            # — moving free size is capped at 512, so two matmuls it is)
            for j in range(2):
                nc.tensor.matmul(
                    psum_u[:, j, :], vp[:, kt, :], et[:, j, :],
                    start=(kt == 0), stop=(kt == NT - 1))

        # finalize one (qh, j): 4 transposes -> one psum tile, batched
        # reciprocal, per-tile normalize mul. Returns list of closures so
        # the emission can be spread through the next qh's loop.
        def finalize_steps(qh, j, psum_u_j, on_scalar, tail_split=False):
            qc = qh * 2 + j
            state = {}

            def s_copy():
                ut = fin_pool.tile([H + 1, QC], F32, tag="ut", name="ut")
                if on_scalar:
                    nc.scalar.copy(out=ut, in_=psum_u_j)
                else:
                    nc.vector.tensor_copy(ut, psum_u_j)
                state["ut"] = ut

            def s_trans():
                pso = ps_s.tile([P, 4, H + 1], F32, tag="ss", name="pso")
                for t in range(4):
                    nc.tensor.transpose(
                        pso[:, t, :], state["ut"][:, t * P:(t + 1) * P],
                        ident[:H + 1, :H + 1])
                rcp = fin_pool.tile([P, 4, 1], F32, tag="rcp", name="rcp")
                nc.vector.reciprocal(rcp, pso[:, :, H:H + 1])
                state["pso"] = pso
                state["rcp"] = rcp

            def s_norm():
                pso, rcp = state["pso"], state["rcp"]
                ot = fin_pool.tile([P, 4, H], BF16, tag="ot", name="ot")
                for t in range(4):
                    # split the 4 muls across both engines when this is
                    # kernel-tail work (scalar.mul is ~403 ns vs DVE ~284;
                    # serializing 4 on one engine is the tail's critical path)
                    use_scalar = on_scalar if not tail_split else (t % 2 == 0)
                    if use_scalar:
                        nc.scalar.mul(ot[:, t, :], pso[:, t, 0:H],
                                      rcp[:, t, :])
                    else:
                        nc.vector.tensor_scalar_mul(ot[:, t, :],
                                                    pso[:, t, 0:H],
                                                    rcp[:, t, :])
                nc.sync.dma_start(
                    out=out_ext.rearrange("p (a h) -> p a h", h=H)[
                        :, qc * 4:qc * 4 + 4, :],
                    in_=ot)

            return [s_copy, s_trans, s_norm]

        # ONE continuous 32-iteration pipeline across both q-halves: the
        # scores/exp stream never drains at the qh boundary (the split-loop
        # version lost ~2.5 us there: 2 PV-only drain iterations starved the
        # exp engines, then qh1 refilled the lookahead from empty). qh0's
        # finalize is queued as posts right after its last PV; the scheduler
        # hoists the accumulator copies so qh1's PVs wait only ~one copy.
        post = {}  # gi -> list of finalize closures
        psum_u_of = {}
        for qh in range(2):
            psum_u_of[qh] = ps_u_pool.tile([H + 1, 2, QC], F32, tag="pu",
                                           name=f"psum_u{qh}")
        ets = {}
        NTOT = 2 * NT
        # blocks of 2 kt: both score pairs back-to-back (PE stays in the
        # split-tile mode, the second pair's LDWEIGHTS hides under the
        # first's stream), then both PV pairs (full-height mode) — two PE
        # array-mode transitions per 2 iterations instead of four.
        def emit_block_scores(g0):
            for gi in (g0, g0 + 1):
                if gi < NTOT:
                    qh, kt = divmod(gi, NT)
                    f = filler.get(gi)
                    if f:
                        f()
                    for step in post.pop(gi, []):
                        step()
                    ets[gi] = emit_scores_exp(qh, kt)

        def emit_block_pv(g0):
            for gi in (g0, g0 + 1):
                pi = gi - LOOKAHEAD
                if pi < 0:
                    continue
                pqh, pkt = divmod(pi, NT)
                emit_pv(psum_u_of[pqh], pkt, ets.pop(pi))
                if pi == NT - 1:
                    # qh0 accumulators complete: both j-copies fire in the
                    # NEXT iteration on opposite engines so the single
                    # 2-bank pu slot frees fast for qh1's first PV.
                    for j in range(2):
                        steps = finalize_steps(0, j, psum_u_of[0][:, j, :],
                                               j == 0)
                        keys = [gi + 1, gi + 2, gi + 4] if j == 0 else \
                               [gi + 1, gi + 3, gi + 6]
                        for k, stp in zip(keys, steps):
                            post.setdefault(k, []).append(stp)

        # fixed block order: scores pair, then PV pair. (An alternating
        # PV-first/scores-first order with lookahead 4 halves the PE
        # array-mode reconfigs but turns the loop exp-bound — measured
        # slower.)
        for g0 in range(0, NTOT + LOOKAHEAD, 2):
            emit_block_scores(g0)
            emit_block_pv(g0)
        # tail: interleave j0 (scalar) / j1 (vector) step-by-step so both
        # engines drain the last two accumulators in parallel
        s0 = finalize_steps(1, 0, psum_u_of[1][:, 0, :], True,
                            tail_split=True)
        s1 = finalize_steps(1, 1, psum_u_of[1][:, 1, :], False,
                            tail_split=True)
        for a, b in zip(s0, s1):
            a()
            b()


def make_in_maps(inputs):
    x = np.ascontiguousarray(inputs["x"], dtype=np.float32)
    wk = np.ascontiguousarray(inputs["Wk"], dtype=np.float32)
    wq = np.ascontiguousarray(inputs["Wq"], dtype=np.float32)
    wv = np.ascontiguousarray(inputs["Wv"], dtype=np.float32)
    assert x.shape == (B, S, D)
    bf = ml_dtypes.bfloat16
    xt = np.ascontiguousarray(
        x.reshape(B, NSTRIP, QC, NCH, P).transpose(0, 4, 1, 3, 2)
    ).astype(bf).reshape(B, P, NSTRIP * NCH * QC)
    wqk = np.concatenate([wq, wk], axis=1)  # [768, 128]
    wqk_h = np.ascontiguousarray(
        wqk.reshape(NCH, P, P).transpose(1, 0, 2)).astype(bf).reshape(P, NCH * P)
    wkq = np.concatenate([wk, wq], axis=1)  # [768, 128] swapped packing
    wkq_h = np.ascontiguousarray(
        wkq.reshape(NCH, P, P).transpose(1, 0, 2)).astype(bf).reshape(P, NCH * P)
    wv_h = np.ascontiguousarray(
        wv.reshape(NCH, P, H).transpose(1, 0, 2)).astype(bf).reshape(P, NCH * H)
    return [{"x": xt[b], "wqk": wqk_h, "wkq": wkq_h, "wv": wv_h}
            for b in range(B)]


_cached_nc = None


def kernel(**inputs):
    global _cached_nc
    if _cached_nc is None:
        _cached_nc = build_kernel()
    nc = _cached_nc
    in_maps = make_in_maps(inputs)
    res = run_bass_kernel_spmd(nc, in_maps, core_ids=list(range(N_CORES)))
    # device out is [p, a, h] with s = a*128 + p; untangle per core
    return np.stack(
        [res.results[i]["out"].astype(np.float32).reshape(P, S // P, H)
         .transpose(1, 0, 2).reshape(S, H) for i in range(N_CORES)], axis=0)

